# revision 19
# baseline (speedup 1.0000x reference)
"""Trainium2 Bass kernel for nn_Attn: additive-attention scores + softmax.

Reference computation (S=512, B=64, H=1024):
    e = relu(concat([hidden bcast, enc], -1) @ Wa^T + ba)      # (S,B,H)
    score = (log(S)/sqrt(H)) * (e @ Ws^T)[...,0]               # (S,B)
    attn = softmax(score.T + pe  with seq_mask -> -1e12, axis=S)  # (B,1,S)

Strategy: data-parallel over B across 8 cores (8 batches each). The concat
splits algebraically: e = relu(enc @ Wa2^T + c[b]) with c = hidden @ Wa1^T + ba
computed once per batch (tiny). Per core the big matmul is (8*512, 1024) @
(1024, 1024), done in e^T orientation (h on partitions, s on free) so the
per-batch bias c fuses into the ACT relu as a per-partition bias and the Ws
reduction is an M=1 matmul on the tensor engine. All matmuls use float32r
(full-rate fp32, ~tf32 mantissa). Host side only reshapes/transposes inputs.
"""
import math
import sys

sys.path.insert(0, "/opt/trn_rl_repo")

import numpy as np

import concourse.bacc as bacc
import concourse.bass as bass
import concourse.mybir as mybir
import concourse.tile as tile
from concourse.bass_utils import run_bass_kernel_spmd

S, B, H = 512, 64, 1024
NCORES = 8
BLOC = B // NCORES          # 8 batches per core
KT = H // 128               # 8 contraction tiles
HT = H // 128               # 8 h-output tiles
SCALE = math.log(S) / math.sqrt(H)

F32R = mybir.dt.float32r
F32 = mybir.dt.float32
U8 = mybir.dt.uint8
AF = mybir.ActivationFunctionType


def build_nc(reps=1):
    """reps>1 wraps the whole body in a hardware loop — used only for timing."""
    nc = bacc.Bacc("TRN2", target_bir_lowering=False, debug=False,
                   num_devices=NCORES)
    xt = nc.dram_tensor("xt", [BLOC, H, S], F32R, kind="ExternalInput").ap()
    # weights pre-tiled on host: [k, h, 128f, 128h] so each (k,h) block is a
    # contiguous 64KB DMA and MM1/cT can start as soon as their slice lands
    wa2t = nc.dram_tensor("wa2t", [KT, HT, 128, 128], F32R,
                          kind="ExternalInput").ap()
    wa1t = nc.dram_tensor("wa1t", [KT, HT, 128, 128], F32R,
                          kind="ExternalInput").ap()
    ht = nc.dram_tensor("ht", [H, BLOC], F32R, kind="ExternalInput").ap()
    # masked Ws^T layout: wstm[p, h*16+8] = Ws[h*128+p], else 0.  MM2 for
    # (h, b) uses the (128, 8) slice [h*16+8-b : h*16+16-b] whose only
    # nonzero column lands at position b -> scores write psum partition b.
    wstm = nc.dram_tensor("wstm", [128, 16 * HT], F32R, kind="ExternalInput").ap()
    ba = nc.dram_tensor("ba", [H, 1], F32, kind="ExternalInput").ap()
    ped = nc.dram_tensor("ped", [BLOC, S], F32, kind="ExternalInput").ap()
    msk = nc.dram_tensor("msk", [BLOC, S], U8, kind="ExternalInput").ap()
    outp = nc.dram_tensor("out", [BLOC, S], F32, kind="ExternalOutput").ap()

    with tile.TileContext(nc) as tc:
        with tc.tile_pool(name="wpool", bufs=1) as wpool, \
             tc.tile_pool(name="xpool", bufs=3) as xpool, \
             tc.tile_pool(name="epool", bufs=5) as epool, \
             tc.tile_pool(name="spool", bufs=1) as spool, \
             tc.tile_pool(name="eps", bufs=4, space="PSUM") as eps, \
             tc.tile_pool(name="sps", bufs=2, space="PSUM") as sps, \
             tc.tile_pool(name="cps", bufs=2, space="PSUM") as cps:

          def emit_body():
            # ---- chunk-0 inputs + h=0 weight slices first: PE starts ASAP ----
            x_sb = []
            for k in range(KT):
                x = xpool.tile([128, S], F32R, tag=f"xt_{k}")
                nc.sync.dma_start(x[:], xt[0, k * 128:(k + 1) * 128, :])
                x_sb.append(x)
            ht_sb = []
            for k in range(KT):
                t = wpool.tile([128, BLOC], F32R, tag=f"ht_{k}")
                nc.sync.dma_start(t[:], ht[k * 128:(k + 1) * 128, :])
                ht_sb.append(t)
            ba_sb = wpool.tile([128, HT], F32, tag="ba")
            nc.sync.dma_start(ba_sb[:], ba.rearrange("(k p) o -> p (k o)", p=128))
            wstm_sb = wpool.tile([128, 16 * HT], F32R, tag="wstm")
            nc.sync.dma_start(wstm_sb[:], wstm)

            wa2_sb = []
            for k in range(KT):
                w2 = wpool.tile([128, H], F32R, tag=f"wa2_{k}")
                nc.sync.dma_start(w2[:].rearrange("p (h q) -> p h q", q=128),
                                  wa2t[k].rearrange("h p q -> p h q"))
                wa2_sb.append(w2)
            wa1_sb = []
            for k in range(KT):
                w1 = wpool.tile([128, H], F32R, tag=f"wa1_{k}")
                nc.sync.dma_start(w1[:].rearrange("p (h q) -> p h q", q=128),
                                  wa1t[k].rearrange("h p q -> p h q"))
                wa1_sb.append(w1)

            # epilogue inputs
            ped_sb = spool.tile([BLOC, S], F32, tag="ped")
            nc.sync.dma_start(ped_sb[:], ped)
            msk_sb = spool.tile([BLOC, S], U8, tag="msk")
            nc.sync.dma_start(msk_sb[:], msk)
            negbig = spool.tile([BLOC, S], F32, tag="negbig")
            nc.vector.memset(negbig[:], -1e12)

            def emit_ct(h):
                # cT[h] = (Wa1 @ hidden^T + ba) h-tile -> (128, BLOC)
                cp = cps.tile([128, BLOC], F32, tag="cps")
                for k in range(KT):
                    nc.tensor.matmul(cp[:], wa1_sb[k][:, h * 128:(h + 1) * 128], ht_sb[k][:],
                                     start=(k == 0), stop=(k == KT - 1))
                ct = wpool.tile([128, BLOC], F32, tag=f"ct_{h}")
                nc.vector.tensor_scalar_add(ct[:], cp[:], ba_sb[:, h:h + 1])
                return ct

            # ---- main loop over local batches ----
            ct_sb = [emit_ct(h) for h in range(HT)]
            spsum = sps.tile([BLOC, S], F32, tag="sp")  # one bank, all scores
            deferred = []  # [(h, e_tile, b)] emitted 2 blocks behind
            for b in range(BLOC):
                if b > 0:
                    x_sb = []
                    for k in range(KT):
                        x = xpool.tile([128, S], F32R, tag=f"xt_{k}")
                        nc.sync.dma_start(x[:], xt[b, k * 128:(k + 1) * 128, :])
                        x_sb.append(x)
                for h in range(HT):
                    ep = eps.tile([128, S], F32, tag="ep")
                    for k in range(KT):
                        nc.tensor.matmul(ep[:], wa2_sb[k][:, h * 128:(h + 1) * 128], x_sb[k][:],
                                         start=(k == 0), stop=(k == KT - 1))
                    e_sb = epool.tile([128, S], F32R, tag="e")
                    nc.scalar.activation(e_sb[:], ep[:], AF.Relu,
                                         bias=ct_sb[h][:, b:b + 1], scale=1.0)
                    # emit score matmuls two h-blocks behind: PE stays well
                    # ahead of the ACT relu dependency
                    deferred.append((h, e_sb, b))
                    if len(deferred) > 2:
                        dh, de, db = deferred.pop(0)
                        nc.tensor.matmul(
                            spsum[:], wstm_sb[:, dh * 16 + 8 - db:dh * 16 + 16 - db],
                            de[:], start=(dh == 0 and db == 0),
                            stop=(dh == HT - 1 and db == BLOC - 1))
            for dh, de, db in deferred:
                nc.tensor.matmul(spsum[:], wstm_sb[:, dh * 16 + 8 - db:dh * 16 + 16 - db],
                                 de[:], start=(dh == 0 and db == 0),
                                 stop=(dh == HT - 1 and db == BLOC - 1))

            # ---- epilogue: t = scores + pe/SCALE ; mask ; softmax(SCALE*t) ----
            t_sb = spool.tile([BLOC, S], F32, tag="t")
            nc.vector.tensor_tensor(out=t_sb[:], in0=spsum[:], in1=ped_sb[:],
                                    op=mybir.AluOpType.add)
            nc.vector.copy_predicated(t_sb[:], msk_sb[:], negbig[:])
            nmax = spool.tile([BLOC, 1], F32, tag="nmax")
            nc.vector.tensor_reduce(out=nmax[:], in_=t_sb[:],
                                    op=mybir.AluOpType.max,
                                    axis=mybir.AxisListType.X, negate=True)
            nmax_s = spool.tile([BLOC, 1], F32, tag="nmax_s")
            nc.vector.tensor_scalar_mul(nmax_s[:], nmax[:], SCALE)
            u_sb = spool.tile([BLOC, S], F32, tag="u")
            esum = spool.tile([BLOC, 1], F32, tag="esum")
            nc.scalar.activation(u_sb[:], t_sb[:], AF.Exp, bias=nmax_s[:],
                                 scale=SCALE, accum_out=esum[:])
            rcp = spool.tile([BLOC, 1], F32, tag="rcp")
            nc.vector.reciprocal(rcp[:], esum[:])
            o_sb = spool.tile([BLOC, S], F32, tag="o")
            nc.vector.tensor_scalar_mul(o_sb[:], u_sb[:], rcp[:])
            nc.sync.dma_start(outp, o_sb[:])

          if reps == 1:
              emit_body()
          else:
              from concourse.engine_type import EngineType
              with tc.For_i(0, reps, 1, hint_engines=(EngineType.PE,)):
                  emit_body()

    nc.compile()
    return nc


def make_in_maps(hidden, encoder_outputs, pe, seq_mask, Wa, ba, Ws):
    """Host-side sharding + layout prep (transposes only, no math beyond pe/SCALE)."""
    hidden = np.asarray(hidden, dtype=np.float32)
    enc = np.asarray(encoder_outputs, dtype=np.float32)
    pe = np.asarray(pe, dtype=np.float32)
    seq_mask = np.asarray(seq_mask)
    Wa = np.asarray(Wa, dtype=np.float32)
    ba = np.asarray(ba, dtype=np.float32)
    Ws = np.asarray(Ws, dtype=np.float32)

    def tile_weights(w):
        # (H, H) f-major -> (KT, HT, 128, 128) contiguous blocks
        return np.ascontiguousarray(
            w.T.reshape(KT, 128, HT, 128).transpose(0, 2, 1, 3))

    wa1t = tile_weights(Wa[:, :H])
    wa2t = tile_weights(Wa[:, H:])
    wstm = np.zeros((128, 16 * HT), dtype=np.float32)
    for h in range(HT):
        wstm[:, h * 16 + 8] = Ws[0, h * 128:(h + 1) * 128]
    ba_col = np.ascontiguousarray(ba.reshape(H, 1))
    ped_all = (pe / np.float32(SCALE)).astype(np.float32)
    msk_all = seq_mask.astype(np.uint8)

    in_maps = []
    for c in range(NCORES):
        bsl = slice(c * BLOC, (c + 1) * BLOC)
        xt = np.ascontiguousarray(enc[:, bsl, :].transpose(1, 2, 0))  # (BLOC,H,S)
        ht = np.ascontiguousarray(hidden[0, bsl, :].T)                # (H, BLOC)
        in_maps.append({
            "xt": xt, "wa2t": wa2t, "wa1t": wa1t, "ht": ht, "wstm": wstm,
            "ba": ba_col, "ped": np.ascontiguousarray(ped_all[bsl]),
            "msk": np.ascontiguousarray(msk_all[bsl]),
        })
    return in_maps


_NC_CACHE = None


def kernel(hidden, encoder_outputs, pe, seq_mask, Wa, ba, Ws):
    global _NC_CACHE
    if _NC_CACHE is None:
        _NC_CACHE = build_nc()
    nc = _NC_CACHE
    in_maps = make_in_maps(hidden, encoder_outputs, pe, seq_mask, Wa, ba, Ws)
    res = run_bass_kernel_spmd(nc, in_maps, list(range(NCORES)))
    attn = np.concatenate([res.results[c]["out"] for c in range(NCORES)], axis=0)
    return attn[:, None, :].astype(np.float32)


# revision 20
# speedup vs baseline: 1.0040x; 1.0040x over previous
"""Trainium2 Bass kernel for nn_Attn: additive-attention scores + softmax.

Reference computation (S=512, B=64, H=1024):
    e = relu(concat([hidden bcast, enc], -1) @ Wa^T + ba)      # (S,B,H)
    score = (log(S)/sqrt(H)) * (e @ Ws^T)[...,0]               # (S,B)
    attn = softmax(score.T + pe  with seq_mask -> -1e12, axis=S)  # (B,1,S)

Strategy: data-parallel over B across 8 cores (8 batches each). The concat
splits algebraically: e = relu(enc @ Wa2^T + c[b]) with c = hidden @ Wa1^T + ba
computed once per batch (tiny). Per core the big matmul is (8*512, 1024) @
(1024, 1024), done in e^T orientation (h on partitions, s on free) so the
per-batch bias c fuses into the ACT relu as a per-partition bias and the Ws
reduction is an M=1 matmul on the tensor engine. All matmuls use float32r
(full-rate fp32, ~tf32 mantissa). Host side only reshapes/transposes inputs.
"""
import math
import sys

sys.path.insert(0, "/opt/trn_rl_repo")

import numpy as np

import concourse.bacc as bacc
import concourse.bass as bass
import concourse.mybir as mybir
import concourse.tile as tile
from concourse.bass_utils import run_bass_kernel_spmd

S, B, H = 512, 64, 1024
NCORES = 8
BLOC = B // NCORES          # 8 batches per core
KT = H // 128               # 8 contraction tiles
HT = H // 128               # 8 h-output tiles
SCALE = math.log(S) / math.sqrt(H)

F32R = mybir.dt.float32r
F32 = mybir.dt.float32
U8 = mybir.dt.uint8
AF = mybir.ActivationFunctionType


def build_nc(reps=1):
    """reps>1 wraps the whole body in a hardware loop — used only for timing."""
    nc = bacc.Bacc("TRN2", target_bir_lowering=False, debug=False,
                   num_devices=NCORES)
    xt = nc.dram_tensor("xt", [BLOC, H, S], F32R, kind="ExternalInput").ap()
    # weights pre-tiled on host: [k, h, 128f, 128h] so each (k,h) block is a
    # contiguous 64KB DMA and MM1/cT can start as soon as their slice lands
    wa2t = nc.dram_tensor("wa2t", [KT, HT, 128, 128], F32R,
                          kind="ExternalInput").ap()
    wa1t = nc.dram_tensor("wa1t", [KT, HT, 128, 128], F32R,
                          kind="ExternalInput").ap()
    ht = nc.dram_tensor("ht", [H, BLOC], F32R, kind="ExternalInput").ap()
    # masked Ws^T layout: wstm[p, h*16+8] = Ws[h*128+p], else 0.  MM2 for
    # (h, b) uses the (128, 8) slice [h*16+8-b : h*16+16-b] whose only
    # nonzero column lands at position b -> scores write psum partition b.
    wstm = nc.dram_tensor("wstm", [128, 16 * HT], F32R, kind="ExternalInput").ap()
    ba = nc.dram_tensor("ba", [H, 1], F32, kind="ExternalInput").ap()
    ped = nc.dram_tensor("ped", [BLOC, S], F32, kind="ExternalInput").ap()
    msk = nc.dram_tensor("msk", [BLOC, S], U8, kind="ExternalInput").ap()
    outp = nc.dram_tensor("out", [BLOC, S], F32, kind="ExternalOutput").ap()

    with tile.TileContext(nc) as tc:
        with tc.tile_pool(name="wpool", bufs=1) as wpool, \
             tc.tile_pool(name="xpool", bufs=2) as xpool, \
             tc.tile_pool(name="epool", bufs=3) as epool, \
             tc.tile_pool(name="spool", bufs=1) as spool, \
             tc.tile_pool(name="eps", bufs=3, space="PSUM") as eps, \
             tc.tile_pool(name="sps", bufs=2, space="PSUM") as sps, \
             tc.tile_pool(name="cps", bufs=2, space="PSUM") as cps:

          def emit_body():
            # ---- chunk-0 inputs + h=0 weight slices first: PE starts ASAP ----
            x_sb = []
            for k in range(KT):
                x = xpool.tile([128, S], F32R, tag=f"xt_{k}")
                nc.sync.dma_start(x[:], xt[0, k * 128:(k + 1) * 128, :])
                x_sb.append(x)
            ht_sb = []
            for k in range(KT):
                t = wpool.tile([128, BLOC], F32R, tag=f"ht_{k}")
                nc.sync.dma_start(t[:], ht[k * 128:(k + 1) * 128, :])
                ht_sb.append(t)
            ba_sb = wpool.tile([128, HT], F32, tag="ba")
            nc.sync.dma_start(ba_sb[:], ba.rearrange("(k p) o -> p (k o)", p=128))
            wstm_sb = wpool.tile([128, 16 * HT], F32R, tag="wstm")
            nc.sync.dma_start(wstm_sb[:], wstm)

            wa2_sb = []
            for k in range(KT):
                w2 = wpool.tile([128, H], F32R, tag=f"wa2_{k}")
                nc.sync.dma_start(w2[:].rearrange("p (h q) -> p h q", q=128),
                                  wa2t[k].rearrange("h p q -> p h q"))
                wa2_sb.append(w2)
            wa1_sb = []
            for k in range(KT):
                w1 = wpool.tile([128, H], F32R, tag=f"wa1_{k}")
                nc.sync.dma_start(w1[:].rearrange("p (h q) -> p h q", q=128),
                                  wa1t[k].rearrange("h p q -> p h q"))
                wa1_sb.append(w1)

            # epilogue inputs
            ped_sb = spool.tile([BLOC, S], F32, tag="ped")
            nc.sync.dma_start(ped_sb[:], ped)
            msk_sb = spool.tile([BLOC, S], U8, tag="msk")
            nc.sync.dma_start(msk_sb[:], msk)
            negbig = spool.tile([BLOC, S], F32, tag="negbig")
            nc.vector.memset(negbig[:], -1e12)

            def emit_ct(h):
                # cT[h] = (Wa1 @ hidden^T + ba) h-tile -> (128, BLOC)
                cp = cps.tile([128, BLOC], F32, tag="cps")
                for k in range(KT):
                    nc.tensor.matmul(cp[:], wa1_sb[k][:, h * 128:(h + 1) * 128], ht_sb[k][:],
                                     start=(k == 0), stop=(k == KT - 1))
                ct = wpool.tile([128, BLOC], F32, tag=f"ct_{h}")
                nc.vector.tensor_scalar_add(ct[:], cp[:], ba_sb[:, h:h + 1])
                return ct

            # ---- main loop over local batches ----
            ct_sb = [emit_ct(h) for h in range(HT)]
            spsum = sps.tile([BLOC, S], F32, tag="sp")  # one bank, all scores
            deferred = []  # [(h, e_tile, b)] emitted one block behind
            for b in range(BLOC):
                if b > 0:
                    x_sb = []
                    for k in range(KT):
                        x = xpool.tile([128, S], F32R, tag=f"xt_{k}")
                        nc.sync.dma_start(x[:], xt[b, k * 128:(k + 1) * 128, :])
                        x_sb.append(x)
                for h in range(HT):
                    ep = eps.tile([128, S], F32, tag="ep")
                    for k in range(KT):
                        nc.tensor.matmul(ep[:], wa2_sb[k][:, h * 128:(h + 1) * 128], x_sb[k][:],
                                         start=(k == 0), stop=(k == KT - 1))
                    e_sb = epool.tile([128, S], F32R, tag="e")
                    nc.scalar.activation(e_sb[:], ep[:], AF.Relu,
                                         bias=ct_sb[h][:, b:b + 1], scale=1.0)
                    # emit score matmuls one h-block behind: PE stays ahead
                    # of the ACT relu dependency
                    deferred.append((h, e_sb, b))
                    if len(deferred) > 1:
                        dh, de, db = deferred.pop(0)
                        nc.tensor.matmul(
                            spsum[:], wstm_sb[:, dh * 16 + 8 - db:dh * 16 + 16 - db],
                            de[:], start=(dh == 0 and db == 0),
                            stop=(dh == HT - 1 and db == BLOC - 1))
            for dh, de, db in deferred:
                nc.tensor.matmul(spsum[:], wstm_sb[:, dh * 16 + 8 - db:dh * 16 + 16 - db],
                                 de[:], start=(dh == 0 and db == 0),
                                 stop=(dh == HT - 1 and db == BLOC - 1))

            # ---- epilogue: t = scores + pe/SCALE ; mask ; softmax(SCALE*t) ----
            t_sb = spool.tile([BLOC, S], F32, tag="t")
            nc.vector.tensor_tensor(out=t_sb[:], in0=spsum[:], in1=ped_sb[:],
                                    op=mybir.AluOpType.add)
            nc.vector.copy_predicated(t_sb[:], msk_sb[:], negbig[:])
            nmax = spool.tile([BLOC, 1], F32, tag="nmax")
            nc.vector.tensor_reduce(out=nmax[:], in_=t_sb[:],
                                    op=mybir.AluOpType.max,
                                    axis=mybir.AxisListType.X, negate=True)
            nmax_s = spool.tile([BLOC, 1], F32, tag="nmax_s")
            nc.vector.tensor_scalar_mul(nmax_s[:], nmax[:], SCALE)
            u_sb = spool.tile([BLOC, S], F32, tag="u")
            esum = spool.tile([BLOC, 1], F32, tag="esum")
            nc.scalar.activation(u_sb[:], t_sb[:], AF.Exp, bias=nmax_s[:],
                                 scale=SCALE, accum_out=esum[:])
            rcp = spool.tile([BLOC, 1], F32, tag="rcp")
            nc.vector.reciprocal(rcp[:], esum[:])
            o_sb = spool.tile([BLOC, S], F32, tag="o")
            nc.vector.tensor_scalar_mul(o_sb[:], u_sb[:], rcp[:])
            nc.sync.dma_start(outp, o_sb[:])

          if reps == 1:
              emit_body()
          else:
              from concourse.engine_type import EngineType
              with tc.For_i(0, reps, 1, hint_engines=(EngineType.PE,)):
                  emit_body()

    nc.compile()
    return nc


def make_in_maps(hidden, encoder_outputs, pe, seq_mask, Wa, ba, Ws):
    """Host-side sharding + layout prep (transposes only, no math beyond pe/SCALE)."""
    hidden = np.asarray(hidden, dtype=np.float32)
    enc = np.asarray(encoder_outputs, dtype=np.float32)
    pe = np.asarray(pe, dtype=np.float32)
    seq_mask = np.asarray(seq_mask)
    Wa = np.asarray(Wa, dtype=np.float32)
    ba = np.asarray(ba, dtype=np.float32)
    Ws = np.asarray(Ws, dtype=np.float32)

    def tile_weights(w):
        # (H, H) f-major -> (KT, HT, 128, 128) contiguous blocks
        return np.ascontiguousarray(
            w.T.reshape(KT, 128, HT, 128).transpose(0, 2, 1, 3))

    wa1t = tile_weights(Wa[:, :H])
    wa2t = tile_weights(Wa[:, H:])
    wstm = np.zeros((128, 16 * HT), dtype=np.float32)
    for h in range(HT):
        wstm[:, h * 16 + 8] = Ws[0, h * 128:(h + 1) * 128]
    ba_col = np.ascontiguousarray(ba.reshape(H, 1))
    ped_all = (pe / np.float32(SCALE)).astype(np.float32)
    msk_all = seq_mask.astype(np.uint8)

    in_maps = []
    for c in range(NCORES):
        bsl = slice(c * BLOC, (c + 1) * BLOC)
        xt = np.ascontiguousarray(enc[:, bsl, :].transpose(1, 2, 0))  # (BLOC,H,S)
        ht = np.ascontiguousarray(hidden[0, bsl, :].T)                # (H, BLOC)
        in_maps.append({
            "xt": xt, "wa2t": wa2t, "wa1t": wa1t, "ht": ht, "wstm": wstm,
            "ba": ba_col, "ped": np.ascontiguousarray(ped_all[bsl]),
            "msk": np.ascontiguousarray(msk_all[bsl]),
        })
    return in_maps


_NC_CACHE = None


def kernel(hidden, encoder_outputs, pe, seq_mask, Wa, ba, Ws):
    global _NC_CACHE
    if _NC_CACHE is None:
        _NC_CACHE = build_nc()
    nc = _NC_CACHE
    in_maps = make_in_maps(hidden, encoder_outputs, pe, seq_mask, Wa, ba, Ws)
    res = run_bass_kernel_spmd(nc, in_maps, list(range(NCORES)))
    attn = np.concatenate([res.results[c]["out"] for c in range(NCORES)], axis=0)
    return attn[:, None, :].astype(np.float32)


# revision 22
# speedup vs baseline: 1.0158x; 1.0117x over previous
"""Trainium2 Bass kernel for nn_Attn: additive-attention scores + softmax.

Reference computation (S=512, B=64, H=1024):
    e = relu(concat([hidden bcast, enc], -1) @ Wa^T + ba)      # (S,B,H)
    score = (log(S)/sqrt(H)) * (e @ Ws^T)[...,0]               # (S,B)
    attn = softmax(score.T + pe  with seq_mask -> -1e12, axis=S)  # (B,1,S)

Strategy: data-parallel over B across 8 cores (8 batches each). The concat
splits algebraically: e = relu(enc @ Wa2^T + c[b]) with c = hidden @ Wa1^T + ba
computed once per batch (tiny). Per core the big matmul is (8*512, 1024) @
(1024, 1024), done in e^T orientation (h on partitions, s on free) so the
per-batch bias c fuses into the ACT relu as a per-partition bias and the Ws
reduction is an M=1 matmul on the tensor engine. All matmuls use fp16
(full-rate, fast weight load, ~1e-4 end-to-end error). Host side only reshapes/transposes inputs.
"""
import math
import sys

sys.path.insert(0, "/opt/trn_rl_repo")

import numpy as np

import concourse.bacc as bacc
import concourse.bass as bass
import concourse.mybir as mybir
import concourse.tile as tile
from concourse.bass_utils import run_bass_kernel_spmd

S, B, H = 512, 64, 1024
NCORES = 8
BLOC = B // NCORES          # 8 batches per core
KT = H // 128               # 8 contraction tiles
HT = H // 128               # 8 h-output tiles
SCALE = math.log(S) / math.sqrt(H)

F32R = mybir.dt.float32r
F16 = mybir.dt.float16
F32 = mybir.dt.float32
U8 = mybir.dt.uint8
AF = mybir.ActivationFunctionType


def build_nc(reps=1):
    """reps>1 wraps the whole body in a hardware loop — used only for timing."""
    nc = bacc.Bacc("TRN2", target_bir_lowering=False, debug=False,
                   num_devices=NCORES)
    xt = nc.dram_tensor("xt", [BLOC, H, S], F16, kind="ExternalInput").ap()
    # weights pre-tiled on host: [k, h, 128f, 128h] so each (k,h) block is a
    # contiguous 64KB DMA and MM1/cT can start as soon as their slice lands
    wa2t = nc.dram_tensor("wa2t", [KT, HT, 128, 128], F16,
                          kind="ExternalInput").ap()
    wa1t = nc.dram_tensor("wa1t", [KT, HT, 128, 128], F16,
                          kind="ExternalInput").ap()
    ht = nc.dram_tensor("ht", [H, BLOC], F16, kind="ExternalInput").ap()
    # masked Ws^T layout: wstm[p, h*16+8] = Ws[h*128+p], else 0.  MM2 for
    # (h, b) uses the (128, 8) slice [h*16+8-b : h*16+16-b] whose only
    # nonzero column lands at position b -> scores write psum partition b.
    wstm = nc.dram_tensor("wstm", [128, 16 * HT], F16, kind="ExternalInput").ap()
    ba = nc.dram_tensor("ba", [H, 1], F32, kind="ExternalInput").ap()
    ped = nc.dram_tensor("ped", [BLOC, S], F32, kind="ExternalInput").ap()
    msk = nc.dram_tensor("msk", [BLOC, S], U8, kind="ExternalInput").ap()
    outp = nc.dram_tensor("out", [BLOC, S], F32, kind="ExternalOutput").ap()

    with tile.TileContext(nc) as tc:
        with tc.tile_pool(name="wpool", bufs=1) as wpool, \
             tc.tile_pool(name="xpool", bufs=2) as xpool, \
             tc.tile_pool(name="epool", bufs=3) as epool, \
             tc.tile_pool(name="spool", bufs=1) as spool, \
             tc.tile_pool(name="eps", bufs=3, space="PSUM") as eps, \
             tc.tile_pool(name="sps", bufs=2, space="PSUM") as sps, \
             tc.tile_pool(name="cps", bufs=2, space="PSUM") as cps:

          def emit_body():
            # ---- chunk-0 inputs + h=0 weight slices first: PE starts ASAP ----
            x_sb = []
            for k in range(KT):
                x = xpool.tile([128, S], F16, tag=f"xt_{k}")
                nc.sync.dma_start(x[:], xt[0, k * 128:(k + 1) * 128, :])
                x_sb.append(x)
            ht_sb = []
            for k in range(KT):
                t = wpool.tile([128, BLOC], F16, tag=f"ht_{k}")
                nc.sync.dma_start(t[:], ht[k * 128:(k + 1) * 128, :])
                ht_sb.append(t)
            ba_sb = wpool.tile([128, HT], F32, tag="ba")
            nc.sync.dma_start(ba_sb[:], ba.rearrange("(k p) o -> p (k o)", p=128))
            wstm_sb = wpool.tile([128, 16 * HT], F16, tag="wstm")
            nc.sync.dma_start(wstm_sb[:], wstm)

            wa2_sb = []
            for k in range(KT):
                w2 = wpool.tile([128, H], F16, tag=f"wa2_{k}")
                nc.sync.dma_start(w2[:].rearrange("p (h q) -> p h q", q=128),
                                  wa2t[k].rearrange("h p q -> p h q"))
                wa2_sb.append(w2)
            wa1_sb = []
            for k in range(KT):
                w1 = wpool.tile([128, H], F16, tag=f"wa1_{k}")
                nc.sync.dma_start(w1[:].rearrange("p (h q) -> p h q", q=128),
                                  wa1t[k].rearrange("h p q -> p h q"))
                wa1_sb.append(w1)

            # epilogue inputs
            ped_sb = spool.tile([BLOC, S], F32, tag="ped")
            nc.sync.dma_start(ped_sb[:], ped)
            msk_sb = spool.tile([BLOC, S], U8, tag="msk")
            nc.sync.dma_start(msk_sb[:], msk)
            negbig = spool.tile([BLOC, S], F32, tag="negbig")
            nc.vector.memset(negbig[:], -1e12)

            def emit_ct(h):
                # cT[h] = (Wa1 @ hidden^T + ba) h-tile -> (128, BLOC)
                cp = cps.tile([128, BLOC], F32, tag="cps")
                for k in range(KT):
                    nc.tensor.matmul(cp[:], wa1_sb[k][:, h * 128:(h + 1) * 128], ht_sb[k][:],
                                     start=(k == 0), stop=(k == KT - 1))
                ct = wpool.tile([128, BLOC], F32, tag=f"ct_{h}")
                nc.vector.tensor_scalar_add(ct[:], cp[:], ba_sb[:, h:h + 1])
                return ct

            # ---- main loop over local batches ----
            ct_sb = [emit_ct(h) for h in range(HT)]
            spsum = sps.tile([BLOC, S], F32, tag="sp")  # one bank, all scores
            deferred = []  # [(h, e_tile, b)] emitted one block behind
            for b in range(BLOC):
                if b > 0:
                    x_sb = []
                    for k in range(KT):
                        x = xpool.tile([128, S], F16, tag=f"xt_{k}")
                        nc.sync.dma_start(x[:], xt[b, k * 128:(k + 1) * 128, :])
                        x_sb.append(x)
                for h in range(HT):
                    ep = eps.tile([128, S], F32, tag="ep")
                    for k in range(KT):
                        nc.tensor.matmul(ep[:], wa2_sb[k][:, h * 128:(h + 1) * 128], x_sb[k][:],
                                         start=(k == 0), stop=(k == KT - 1))
                    e_sb = epool.tile([128, S], F16, tag="e")
                    nc.scalar.activation(e_sb[:], ep[:], AF.Relu,
                                         bias=ct_sb[h][:, b:b + 1], scale=1.0)
                    # emit score matmuls one h-block behind: PE stays ahead
                    # of the ACT relu dependency
                    deferred.append((h, e_sb, b))
                    if len(deferred) > 1:
                        dh, de, db = deferred.pop(0)
                        nc.tensor.matmul(
                            spsum[:], wstm_sb[:, dh * 16 + 8 - db:dh * 16 + 16 - db],
                            de[:], start=(dh == 0 and db == 0),
                            stop=(dh == HT - 1 and db == BLOC - 1))
            for dh, de, db in deferred:
                nc.tensor.matmul(spsum[:], wstm_sb[:, dh * 16 + 8 - db:dh * 16 + 16 - db],
                                 de[:], start=(dh == 0 and db == 0),
                                 stop=(dh == HT - 1 and db == BLOC - 1))

            # ---- epilogue: t = scores + pe/SCALE ; mask ; softmax(SCALE*t) ----
            t_sb = spool.tile([BLOC, S], F32, tag="t")
            nc.vector.tensor_tensor(out=t_sb[:], in0=spsum[:], in1=ped_sb[:],
                                    op=mybir.AluOpType.add)
            nc.vector.copy_predicated(t_sb[:], msk_sb[:], negbig[:])
            nmax = spool.tile([BLOC, 1], F32, tag="nmax")
            nc.vector.tensor_reduce(out=nmax[:], in_=t_sb[:],
                                    op=mybir.AluOpType.max,
                                    axis=mybir.AxisListType.X, negate=True)
            nmax_s = spool.tile([BLOC, 1], F32, tag="nmax_s")
            nc.vector.tensor_scalar_mul(nmax_s[:], nmax[:], SCALE)
            u_sb = spool.tile([BLOC, S], F32, tag="u")
            esum = spool.tile([BLOC, 1], F32, tag="esum")
            nc.scalar.activation(u_sb[:], t_sb[:], AF.Exp, bias=nmax_s[:],
                                 scale=SCALE, accum_out=esum[:])
            rcp = spool.tile([BLOC, 1], F32, tag="rcp")
            nc.vector.reciprocal(rcp[:], esum[:])
            o_sb = spool.tile([BLOC, S], F32, tag="o")
            nc.vector.tensor_scalar_mul(o_sb[:], u_sb[:], rcp[:])
            nc.sync.dma_start(outp, o_sb[:])

          if reps == 1:
              emit_body()
          else:
              from concourse.engine_type import EngineType
              with tc.For_i(0, reps, 1, hint_engines=(EngineType.PE,)):
                  emit_body()

    nc.compile()
    return nc


def make_in_maps(hidden, encoder_outputs, pe, seq_mask, Wa, ba, Ws):
    """Host-side sharding + layout prep (transposes only, no math beyond pe/SCALE)."""
    hidden = np.asarray(hidden, dtype=np.float32)
    enc = np.asarray(encoder_outputs, dtype=np.float32)
    pe = np.asarray(pe, dtype=np.float32)
    seq_mask = np.asarray(seq_mask)
    Wa = np.asarray(Wa, dtype=np.float32)
    ba = np.asarray(ba, dtype=np.float32)
    Ws = np.asarray(Ws, dtype=np.float32)

    def tile_weights(w):
        # (H, H) f-major -> (KT, HT, 128, 128) contiguous blocks
        return np.ascontiguousarray(
            w.T.reshape(KT, 128, HT, 128).transpose(0, 2, 1, 3))

    wa1t = tile_weights(Wa[:, :H]).astype(np.float16)
    wa2t = tile_weights(Wa[:, H:]).astype(np.float16)
    wstm = np.zeros((128, 16 * HT), dtype=np.float16)
    for h in range(HT):
        wstm[:, h * 16 + 8] = Ws[0, h * 128:(h + 1) * 128]
    ba_col = np.ascontiguousarray(ba.reshape(H, 1))
    ped_all = (pe / np.float32(SCALE)).astype(np.float32)
    msk_all = seq_mask.astype(np.uint8)

    in_maps = []
    for c in range(NCORES):
        bsl = slice(c * BLOC, (c + 1) * BLOC)
        xt = np.ascontiguousarray(
            enc[:, bsl, :].transpose(1, 2, 0).astype(np.float16))  # (BLOC,H,S)
        ht = np.ascontiguousarray(hidden[0, bsl, :].T.astype(np.float16))  # (H, BLOC)
        in_maps.append({
            "xt": xt, "wa2t": wa2t, "wa1t": wa1t, "ht": ht, "wstm": wstm,
            "ba": ba_col, "ped": np.ascontiguousarray(ped_all[bsl]),
            "msk": np.ascontiguousarray(msk_all[bsl]),
        })
    return in_maps


_NC_CACHE = None


def kernel(hidden, encoder_outputs, pe, seq_mask, Wa, ba, Ws):
    global _NC_CACHE
    if _NC_CACHE is None:
        _NC_CACHE = build_nc()
    nc = _NC_CACHE
    in_maps = make_in_maps(hidden, encoder_outputs, pe, seq_mask, Wa, ba, Ws)
    res = run_bass_kernel_spmd(nc, in_maps, list(range(NCORES)))
    attn = np.concatenate([res.results[c]["out"] for c in range(NCORES)], axis=0)
    return attn[:, None, :].astype(np.float32)
